# revision 79
# baseline (speedup 1.0000x reference)
"""BFLinear (block-floating-point linear) Trainium2 kernel — fp8 DoubleRow.

Full problem: x[8192,4096] f32, weight[4096,4096] f32, bias[4096] f32.
  out = set_mantissa(bfp8_32(x) @ bfp8_32(weight).T + bias, 16 bits)

Sharding across 8 NeuronCores: 4 row-blocks of x  x  2 col-blocks of weight.
Each core computes outT_shard [N_C, M_C] fp16, host reassembles/transposes.

Numerical scheme: instead of replicating the reference's 8-bit-mantissa BFP
exactly (which forces bf16 matmuls), each operand is split into a hi/lo pair
of fp8 e4m3 planes at a fixed power-of-2 scale:
    xh = fp8(x*SX),  xl = fp8(x*SX - xh)      (and same for w at SW)
    out ~= (xh@wh + xl@wh + xh@wl) / (SX*SW) + bias
The dropped xl@wl term and the second-level quantization add ~0.7% rel l2 on
top of the reference's own BFP noise (~0.9%), landing at 1.17e-2 — inside
the 2e-2 gate with ~1.7x margin.  The payoff: all three matmul terms run as
fp8 MatmulPerfMode.DoubleRow (256-deep contraction, 0.5 cycles per moving
row), so per-core PE matmul time is 328us where the bit-exact bf16 path
needs 437us.

Per-core pipeline (single Tile program, SPMD, measured 530941 ns):
  1. load: f32 [128,1024] strip-chunks on the SP HWDGE queue.
  2. quantize x: ACT casts the hi plane (Copy with scale), one custom DVE op
     computes the lo residual (Src0*SX - Src1) straight to fp8.
     quantize w: same but hi on gpsimd and both planes written
     byte-interleaved (wh,wl) into one packed u16-grid tile.
  3. transpose to k-major [k = 128*kb + partition] operand layouts:
       - x planes: fp8 PE identity transposes (the PE is otherwise idle
         during the x fill; PSUM holds fp8 at u16 spacing) + strided ACT/DVE
         copies compacting into the resident caches [128, 2, KB, M_C].
       - w: one DMA-xbar transpose per packed chunk straight into streamed
         slots [128, KB, 2*128] — no copies; the DoubleRow stationary reads
         wh/wl via inner stride-2 APs (wl at the odd byte offset), which the
         LdWeights ISA permits (the 16B slot-stride rule only binds the
         outer/slot dim, satisfied by the kb-pair stride).
  4. matmul: per (w-sixteenth e, m-block-pair) one full-bank PSUM
     [128, 2, 256] f32 accumulates 2 x 48 DoubleRow matmuls
     (16 k-supertiles x 3 terms), moving free = (2, 256).
  5. evict (ACT): fp16 with fused bias + 2^-14 descale, stores on the
     gpsimd SWDGE queue (emitted one production step behind their evicts).

Schedule: a short burst of dummy PE transposes warms the pstate ramp, then
a staircase over (e < 4) x m-block-pairs keeps the PE on transposes+waves
while x streams in; the remaining 12 sixteenths stream through a 5-slot
pool, quant chains prefetched ~4 slots ahead of their waves.  Deep evict
(12) and psum (5+3) pools keep the tile framework's buffer-recycle barriers
(EventSemaphores, which serialize whole engine queues) off the critical
path — shallow pools there previously cost >70us in cascaded stalls and
pstate resets.
"""

import re
from contextlib import ExitStack

import numpy as np

import concourse.bass as bass
import concourse.dve_ops as dve_ops
from concourse import bacc
import concourse.tile as tile
from concourse import mybir
from concourse.bass_utils import run_bass_kernel_spmd
from concourse.dve_spec import C0, Spec, Src0, Src1
from concourse.masks import make_identity

P = 128
F32 = mybir.dt.float32
F16 = mybir.dt.float16
FP8 = mybir.dt.float8e4
U16 = mybir.dt.uint16
U32 = mybir.dt.uint32

SX = 32.0      # |x| < 7.5 -> |xh| < 240 (e4m3 max)
SW = 512.0     # |w| < 0.11 -> |wh| < 56.3
OSCALE = 1.0 / (SX * SW)


# --------------------------------------------------------------------------
# custom DVE op: out_fp8 = Src0 * C0 - Src1   (lo-plane residual)
# --------------------------------------------------------------------------
def _lo_ref(in0, in1, s0, s1, imm2):
    g = np.asarray(in0, np.float32)
    h = np.asarray(in1, np.float32)
    return (g * np.float32(s0) - h).astype(np.float32)


def _hi_ref(in0, in1, s0, s1, imm2):
    g = np.asarray(in0, np.float32)
    return (g * np.float32(s0)).astype(np.float32)


def _register_op(name, spec):
    for existing in dve_ops.OPS:
        if existing.name == name:
            return existing
    ver = "v3"
    op = dve_ops.DveOp(name, spec, subdim=False, uops_sha={})
    dve_ops.OPS.append(op)
    dve_ops._SUB_OPCODE_FOR_NAME[name] = (
        dve_ops._CUSTOM_DVE_ROW_BASE + len(dve_ops.OPS) - 1
    )
    dve_ops.CUSTOM_DVE_SPECS[name] = spec
    try:
        op.compile(ver)
    except ValueError as e:
        m = re.search(r'uops_sha\["v3"\]="([0-9a-f]+)"', str(e))
        if not m:
            raise
        op = dve_ops.DveOp(name, spec, subdim=False, uops_sha={ver: m.group(1)})
        dve_ops.OPS[-1] = op
    op.compile(ver)
    return op


LO_OP = _register_op("FP8_LO_RESID_ANT", Spec(body=Src0 * C0 - Src1,
                                              reference=_lo_ref))
HI_OP = _register_op("FP8_HI_SCALE_ANT", Spec(body=Src0 * C0,
                                              reference=_hi_ref))


# --------------------------------------------------------------------------
# program builder
# --------------------------------------------------------------------------
def build_program(M_C, K, N_C, num_devices=1):
    """One SPMD core program: xs [M_C,K], ws [N_C,K], bias_s [N_C]
    -> outT [N_C, M_C] fp16.

    Operand layout (both sides k-major, one k per partition,
    k = 128*kb + partition):
      x planes: [128, 2(plane), KB, M_C] fp8, filled by fp8 PE identity
        transposes (PSUM holds fp8 at u16 spacing; a strided ACT/DVE copy
        compacts into the cache).
      w slots:  [128, KB, 2*128] fp8 with wh/wl byte-interleaved along n,
        filled by the DMA xbar from a (wh,wl)-packed u16 quant tile; the
        DoubleRow stationary reads it with an inner stride-2 AP (wl at the
        odd byte offset), which the ISA allows for LdWeights.
    DoubleRow slot pairs are adjacent kb blocks (stride 16B-aligned as the
    ISA requires); each matmul contracts 256 k at 0.5 cycles/row.
    """
    XT = 1024                   # staged strip k-width
    NKT = K // XT               # k-chunks per strip (4)
    KB = K // P                 # k blocks (32)
    KBT = XT // P               # k blocks per staged chunk (8)
    KS = K // 256               # DoubleRow k-supertiles (16)
    NE = N_C // P               # w sixteenths (16)
    MBW = 256                   # m-block width
    NMB = M_C // MBW            # m-blocks (8)
    FILL_E = min(4, N_C // P)   # w slots resident through x fill
    DR = mybir.MatmulPerfMode.DoubleRow

    nc = bacc.Bacc("TRN2", target_bir_lowering=False, debug=False,
                   enable_asserts=True, num_devices=num_devices)
    xs = nc.dram_tensor("xs", [M_C, K], F32, kind="ExternalInput").ap()
    ws = nc.dram_tensor("ws", [N_C, K], F32, kind="ExternalInput").ap()
    bias_s = nc.dram_tensor("bias_s", [N_C], F32, kind="ExternalInput").ap()
    outT = nc.dram_tensor("outT", [N_C, M_C], F16, kind="ExternalOutput").ap()

    with tile.TileContext(nc) as tc, ExitStack() as ctx:
        stage = ctx.enter_context(tc.tile_pool(name="stage", bufs=4))
        q8 = ctx.enter_context(tc.tile_pool(name="q8", bufs=3))
        xcache = ctx.enter_context(tc.tile_pool(name="xc", bufs=1))
        wpool = ctx.enter_context(tc.tile_pool(name="wp", bufs=FILL_E + 1))
        opool = ctx.enter_context(tc.tile_pool(name="outs", bufs=12))
        cpool = ctx.enter_context(tc.tile_pool(name="consts", bufs=1))
        psum = ctx.enter_context(tc.tile_pool(name="ps", bufs=5, space="PSUM"))
        tpsum = ctx.enter_context(tc.tile_pool(name="tp", bufs=3, space="PSUM"))

        ident = cpool.tile([P, P], FP8)
        make_identity(nc, ident[:])

        # warm the PE pstate ramp: ~4us of dummy transposes while the first
        # input loads are still in flight (full clock needs 3us busy)
        for _wu in range(6):
            wtp = tpsum.tile([P, KBT, 2 * P], FP8, tag="tp", name=f"wu{_wu}")
            for j in range(KBT):
                tpv = wtp[:, j, :]
                tp2 = bass.AP(tensor=tpv.tensor, offset=tpv.offset,
                              ap=[tpv.ap[0], [2, P]])
                nc.tensor.transpose(tp2, ident[:], ident[:])

        # bias staged [P, NE]: col e, part p = bias[e*128 + p]
        bias_sb = cpool.tile([P, NE], F32)
        nc.sync.dma_start(
            out=bias_sb[:],
            in_=bass.AP(tensor=bias_s.tensor, offset=bias_s.offset,
                        ap=[[1, P], [P, NE]]),
        )

        # x plane caches, fully resident: [128 k, plane, KB, M_C] fp8
        x_c = xcache.tile([P, 2, KB, M_C], FP8, name="x_c")
        xh_c, xl_c = x_c[:, 0], x_c[:, 1]
        w_t = {}                # e -> slot tile [P, KB, 2*P]

        def quant_x(src_rows, scale, hi_pool=False):
            """Load one [128, XT] f32 x strip-chunk; hi (ACT or Pool) /
            lo (DVE) into separate contiguous fp8 planes."""
            xt = stage.tile([P, XT], F32, tag="xt")
            nc.sync.dma_start(out=xt[:], in_=src_rows)
            t8 = q8.tile([P, 2, XT], FP8, tag="t8")
            th, tl = t8[:, 0, :], t8[:, 1, :]
            if hi_pool:
                nc.gpsimd.tensor_scalar(out=th, in0=xt[:],
                                        scalar1=float(scale), scalar2=None,
                                        op0=mybir.AluOpType.mult)
            else:
                nc.scalar.activation(out=th, in_=xt[:],
                                     func=mybir.ActivationFunctionType.Copy,
                                     scale=float(scale))
            nc.vector._custom_dve(LO_OP, out=tl, in0=xt[:], in1=th,
                                  s0=float(scale), s1=0.0)
            return th, tl

        def quant_w_packed(src_rows, scale, hi_eng="pool"):
            """Load one [128, XT] f32 w strip-chunk; hi / lo byte-interleaved
            into one packed (wh,wl) u16-grid tile. hi rides the Pool in the
            fill (ACT/DVE are saturated there) and the DVE in steady state
            (shorter chain; Pool keeps only the output stores)."""
            xt = stage.tile([P, XT], F32, tag="xt")
            nc.sync.dma_start(out=xt[:], in_=src_rows)
            t8 = q8.tile([P, 2, XT], FP8, tag="t8")
            b = t8[:].rearrange("p a x -> p (a x)")
            th = bass.AP(tensor=b.tensor, offset=b.offset,
                         ap=[b.ap[0], [2, XT]])
            tl = bass.AP(tensor=b.tensor, offset=b.offset + 1,
                         ap=[b.ap[0], [2, XT]])
            if hi_eng == "pool":
                nc.gpsimd.tensor_scalar(out=th, in0=xt[:],
                                        scalar1=float(scale), scalar2=None,
                                        op0=mybir.AluOpType.mult)
            elif hi_eng == "act":
                nc.scalar.activation(out=th, in_=xt[:],
                                     func=mybir.ActivationFunctionType.Copy,
                                     scale=float(scale))
            else:
                nc.vector._custom_dve(HI_OP, out=th, in0=xt[:],
                                      s0=float(scale), s1=0.0)
            nc.vector._custom_dve(LO_OP, out=tl, in0=xt[:], in1=th,
                                  s0=float(scale), s1=0.0)
            return t8

        def prod_x(mbp):
            """Quantize + fp8-PE-transpose one m-block pair (4 strips x
            full K) into the x plane caches."""
            for s in range(4 * mbp, 4 * mbp + 4):
                for kt in range(NKT):
                    th, tl = quant_x(
                        xs[s * P:(s + 1) * P, kt * XT:(kt + 1) * XT], SX,
                        hi_pool=False)
                    for t, dst, ce in ((th, xh_c, "act"), (tl, xl_c, "dve")):
                        tp = tpsum.tile([P, KBT, 2 * P], FP8, tag="tp")
                        for j in range(KBT):
                            tpv = tp[:, j, :]
                            tp2 = bass.AP(tensor=tpv.tensor,
                                          offset=tpv.offset,
                                          ap=[tpv.ap[0], [2, P]])
                            nc.tensor.transpose(
                                tp2, t[:, j * P:(j + 1) * P], ident[:])
                        f = tp[:]
                        ina = bass.AP(tensor=f.tensor, offset=f.offset,
                                      ap=[f.ap[0], [2 * P, KBT], [2, P]])
                        outa = dst[:, kt * KBT:(kt + 1) * KBT,
                                   s * P:(s + 1) * P]
                        if ce == "dve":
                            nc.vector.tensor_copy(outa, ina)
                        else:
                            nc.scalar.activation(
                                out=outa, in_=ina,
                                func=mybir.ActivationFunctionType.Copy)

        def prod_w(e, hi_eng="pool"):
            """Quantize one w sixteenth packed and xbar it straight into a
            [128 k, KB, 2*128] slot (no copies)."""
            wt = wpool.tile([P, KB, 2 * P], FP8, tag="wt", name=f"wt{e}")
            w_t[e] = wt
            prev = None

            def emit_tp(t8, kt):
                d16 = wt[:].bitcast(U16)[:, kt * KBT:(kt + 1) * KBT, :]
                nc.sync.dma_start_transpose(d16, t8[:].bitcast(U16))

            for kt in range(NKT):
                t8 = quant_w_packed(
                    ws[e * P:(e + 1) * P, kt * XT:(kt + 1) * XT], SW,
                    hi_eng=hi_eng)
                if prev is not None:
                    emit_tp(*prev)
                prev = (t8, kt)
            emit_tp(*prev)

        def mov_ap(cache, ks, mb):
            d = cache
            return bass.AP(tensor=d.tensor,
                           offset=d.offset + 2 * ks * M_C + mb * MBW,
                           ap=[d.ap[0], [M_C, 2], [1, MBW]])

        def stat_ap(e, ks, plane):
            d = w_t[e][:]
            return bass.AP(tensor=d.tensor,
                           offset=d.offset + 2 * ks * (2 * P) + plane,
                           ap=[d.ap[0], [2 * P, 2], [2, P]])

        def wave2(e, mbp):
            """One full-bank psum [128 n, 2x256 m] (m-blocks 2*mbp, 2*mbp+1)
            over full K: 2 x 48 DoubleRow matmuls, single fused evict."""
            ps = psum.tile([P, 2, MBW], F32, tag="ps", name=f"ps_{e}_{mbp}")
            nmm = 3 * KS
            for h in range(2):
                mb = 2 * mbp + h
                i = 0
                for ks in range(KS):
                    for wp_, mc in ((0, xh_c), (0, xl_c), (1, xh_c)):
                        nc.tensor.matmul(
                            ps[:, h, :], stat_ap(e, ks, wp_),
                            mov_ap(mc, ks, mb),
                            start=(i == 0), stop=(i == nmm - 1), perf_mode=DR)
                        i += 1
            ev = opool.tile([P, 2 * MBW], F16, tag="ev")
            nc.scalar.activation(
                out=ev[:], in_=ps[:].rearrange("p a b -> p (a b)"),
                func=mybir.ActivationFunctionType.Identity,
                bias=bias_sb[:, e:e + 1], scale=float(OSCALE),
            )
            pending_stores.append((ev, e, mbp))

        pending_stores = []

        def flush_stores(eng=None):
            for ev, e, mbp in pending_stores:
                (eng or nc.gpsimd).dma_start(
                    out=outT[e * P:(e + 1) * P,
                             mbp * 2 * MBW:(mbp + 1) * 2 * MBW],
                    in_=ev[:])
            pending_stores.clear()

        # ---------------- emission ----------------
        # Fill: staircase over (e < FILL_E) x mbp while x streams in (the
        # fp8 PE transposes keep the otherwise-idle PE busy). Steady: the
        # remaining sixteenths stream through the slot pool, quant chains
        # prefetched a few slots ahead of their waves.
        NMBP = NMB // 2
        emitted = set()
        ready_e = 0
        ready_p = 0

        def emit_ready():
            for e in range(ready_e):
                for p in range(ready_p):
                    if (e, p) not in emitted:
                        emitted.add((e, p))
                        wave2(e, p)

        steps = []
        for i in range(max(FILL_E, NMBP)):
            if i < NMBP:
                steps.append(("x", i))
            if i < FILL_E:
                steps.append(("w", i))
        for kind, idx in steps:
            if kind == "x":
                prod_x(idx)
                ready_p = idx + 1
            else:
                prod_w(idx)
                ready_e = idx + 1
            flush_stores()
            emit_ready()
        nexte = FILL_E
        for e in range(FILL_E, NE):
            while nexte < min(e + 5, NE):
                prod_w(nexte)
                nexte += 1
            flush_stores()
            for p in range(NMBP):
                wave2(e, p)
        flush_stores()

    nc.compile()
    return nc


_PROGRAM_CACHE = {}


def _get_program(M_C, K, N_C):
    key = (M_C, K, N_C)
    if key not in _PROGRAM_CACHE:
        _PROGRAM_CACHE[key] = build_program(M_C, K, N_C)
    return _PROGRAM_CACHE[key]


LAST_RESULTS = None


def kernel(x, weight, bias):
    global LAST_RESULTS
    M_FULL, K = x.shape
    N_FULL = weight.shape[0]
    RB, CB = 4, 2
    M_C, N_C = M_FULL // RB, N_FULL // CB

    nc = _get_program(M_C, K, N_C)

    x = np.asarray(x, np.float32)
    weight = np.asarray(weight, np.float32)
    bias = np.asarray(bias, np.float32)

    in_maps = []
    blocks = []
    for r in range(RB):
        for c in range(CB):
            in_maps.append({
                "xs": np.ascontiguousarray(x[r * M_C:(r + 1) * M_C]),
                "ws": np.ascontiguousarray(weight[c * N_C:(c + 1) * N_C]),
                "bias_s": np.ascontiguousarray(bias[c * N_C:(c + 1) * N_C]),
            })
            blocks.append((r, c))

    import os
    trace = bool(int(os.environ.get("KERNEL_TRACE", "0")))
    res = run_bass_kernel_spmd(nc, in_maps, core_ids=list(range(len(in_maps))),
                               trace=trace)
    LAST_RESULTS = res

    out = np.empty((M_FULL, N_FULL), np.float32)
    for i, (r, c) in enumerate(blocks):
        out[r * M_C:(r + 1) * M_C, c * N_C:(c + 1) * N_C] = \
            res.results[i]["outT"].astype(np.float32).T
    return out


# revision 91
# speedup vs baseline: 1.0129x; 1.0129x over previous
"""BFLinear (block-floating-point linear) Trainium2 kernel — fp8 DoubleRow.

Full problem: x[8192,4096] f32, weight[4096,4096] f32, bias[4096] f32.
  out = set_mantissa(bfp8_32(x) @ bfp8_32(weight).T + bias, 16 bits)

Sharding across 8 NeuronCores: 4 row-blocks of x  x  2 col-blocks of weight.
Each core computes outT_shard [N_C, M_C] fp16, host reassembles/transposes.

Numerical scheme: instead of replicating the reference's 8-bit-mantissa BFP
exactly (which forces bf16 matmuls), each operand is split into a hi/lo pair
of fp8 e4m3 planes at a fixed power-of-2 scale:
    xh = fp8(x*SX),  xl = fp8(x*SX - xh)      (and same for w at SW)
    out ~= (xh@wh + xl@wh + xh@wl) / (SX*SW) + bias
The dropped xl@wl term and the second-level quantization add ~0.7% rel l2 on
top of the reference's own BFP noise (~0.9%), landing at 1.17e-2 — inside
the 2e-2 gate with ~1.7x margin.  The payoff: all three matmul terms run as
fp8 MatmulPerfMode.DoubleRow (256-deep contraction, 0.5 cycles per moving
row), so per-core PE matmul time is 328us where the bit-exact bf16 path
needs 437us.

Per-core pipeline (single Tile program, SPMD, measured 530941 ns):
  1. load: f32 [128,1024] strip-chunks on the SP HWDGE queue.
  2. quantize x: ACT casts the hi plane (Copy with scale), one custom DVE op
     computes the lo residual (Src0*SX - Src1) straight to fp8.
     quantize w: same but hi on gpsimd and both planes written
     byte-interleaved (wh,wl) into one packed u16-grid tile.
  3. transpose to k-major [k = 128*kb + partition] operand layouts:
       - x planes: fp8 PE identity transposes (the PE is otherwise idle
         during the x fill; PSUM holds fp8 at u16 spacing) + strided ACT/DVE
         copies compacting into the resident caches [128, 2, KB, M_C].
       - w: one DMA-xbar transpose per packed chunk straight into streamed
         slots [128, KB, 2*128] — no copies; the DoubleRow stationary reads
         wh/wl via inner stride-2 APs (wl at the odd byte offset), which the
         LdWeights ISA permits (the 16B slot-stride rule only binds the
         outer/slot dim, satisfied by the kb-pair stride).
  4. matmul: per (w-sixteenth e, m-block-pair) one full-bank PSUM
     [128, 2, 256] f32 accumulates 2 x 48 DoubleRow matmuls
     (16 k-supertiles x 3 terms), moving free = (2, 256).
  5. evict (ACT): fp16 with fused bias + 2^-14 descale, stores on the
     gpsimd SWDGE queue (emitted one production step behind their evicts).

Schedule: a short burst of dummy PE transposes warms the pstate ramp, then
a staircase over (e < 4) x m-block-pairs keeps the PE on transposes+waves
while x streams in; the remaining 12 sixteenths stream through a 5-slot
pool, quant chains prefetched ~4 slots ahead of their waves.  Deep evict
(12) and psum (5+3) pools keep the tile framework's buffer-recycle barriers
(EventSemaphores, which serialize whole engine queues) off the critical
path — shallow pools there previously cost >70us in cascaded stalls and
pstate resets.
"""

import re
from contextlib import ExitStack

import numpy as np

import concourse.bass as bass
import concourse.dve_ops as dve_ops
from concourse import bacc
import concourse.tile as tile
from concourse import mybir
from concourse.bass_utils import run_bass_kernel_spmd
from concourse.dve_spec import C0, Spec, Src0, Src1
from concourse.masks import make_identity

P = 128
F32 = mybir.dt.float32
F16 = mybir.dt.float16
FP8 = mybir.dt.float8e4
U16 = mybir.dt.uint16
U32 = mybir.dt.uint32

SX = 32.0      # |x| < 7.5 -> |xh| < 240 (e4m3 max)
SW = 512.0     # |w| < 0.11 -> |wh| < 56.3
OSCALE = 1.0 / (SX * SW)


# --------------------------------------------------------------------------
# custom DVE op: out_fp8 = Src0 * C0 - Src1   (lo-plane residual)
# --------------------------------------------------------------------------
def _lo_ref(in0, in1, s0, s1, imm2):
    g = np.asarray(in0, np.float32)
    h = np.asarray(in1, np.float32)
    return (g * np.float32(s0) - h).astype(np.float32)


def _hi_ref(in0, in1, s0, s1, imm2):
    g = np.asarray(in0, np.float32)
    return (g * np.float32(s0)).astype(np.float32)


def _register_op(name, spec):
    for existing in dve_ops.OPS:
        if existing.name == name:
            return existing
    ver = "v3"
    op = dve_ops.DveOp(name, spec, subdim=False, uops_sha={})
    dve_ops.OPS.append(op)
    dve_ops._SUB_OPCODE_FOR_NAME[name] = (
        dve_ops._CUSTOM_DVE_ROW_BASE + len(dve_ops.OPS) - 1
    )
    dve_ops.CUSTOM_DVE_SPECS[name] = spec
    try:
        op.compile(ver)
    except ValueError as e:
        m = re.search(r'uops_sha\["v3"\]="([0-9a-f]+)"', str(e))
        if not m:
            raise
        op = dve_ops.DveOp(name, spec, subdim=False, uops_sha={ver: m.group(1)})
        dve_ops.OPS[-1] = op
    op.compile(ver)
    return op


LO_OP = _register_op("FP8_LO_RESID_ANT", Spec(body=Src0 * C0 - Src1,
                                              reference=_lo_ref))
HI_OP = _register_op("FP8_HI_SCALE_ANT", Spec(body=Src0 * C0,
                                              reference=_hi_ref))


# --------------------------------------------------------------------------
# program builder
# --------------------------------------------------------------------------
def build_program(M_C, K, N_C, num_devices=1):
    """One SPMD core program: xs [M_C,K], ws [N_C,K], bias_s [N_C]
    -> outT [N_C, M_C] fp16.

    Operand layout (both sides k-major, one k per partition,
    k = 128*kb + partition):
      x planes: [128, 2(plane), KB, M_C] fp8, filled by fp8 PE identity
        transposes (PSUM holds fp8 at u16 spacing; a strided ACT/DVE copy
        compacts into the cache).
      w slots:  [128, KB, 2*128] fp8 with wh/wl byte-interleaved along n,
        filled by the DMA xbar from a (wh,wl)-packed u16 quant tile; the
        DoubleRow stationary reads it with an inner stride-2 AP (wl at the
        odd byte offset), which the ISA allows for LdWeights.
    DoubleRow slot pairs are adjacent kb blocks (stride 16B-aligned as the
    ISA requires); each matmul contracts 256 k at 0.5 cycles/row.
    """
    XT = 1024                   # staged strip k-width
    NKT = K // XT               # k-chunks per strip (4)
    KB = K // P                 # k blocks (32)
    KBT = XT // P               # k blocks per staged chunk (8)
    KS = K // 256               # DoubleRow k-supertiles (16)
    NE = N_C // P               # w sixteenths (16)
    MBW = 256                   # m-block width
    NMB = M_C // MBW            # m-blocks (8)
    FILL_E = min(1, N_C // P)   # w slots resident through x fill
    DR = mybir.MatmulPerfMode.DoubleRow

    nc = bacc.Bacc("TRN2", target_bir_lowering=False, debug=False,
                   enable_asserts=True, num_devices=num_devices)
    xs = nc.dram_tensor("xs", [M_C, K], F32, kind="ExternalInput").ap()
    ws = nc.dram_tensor("ws", [N_C, K], F32, kind="ExternalInput").ap()
    bias_s = nc.dram_tensor("bias_s", [N_C], F32, kind="ExternalInput").ap()
    outT = nc.dram_tensor("outT", [N_C, M_C], F16, kind="ExternalOutput").ap()

    with tile.TileContext(nc) as tc, ExitStack() as ctx:
        stage = ctx.enter_context(tc.tile_pool(name="stage", bufs=4))
        q8 = ctx.enter_context(tc.tile_pool(name="q8", bufs=3))
        xcache = ctx.enter_context(tc.tile_pool(name="xc", bufs=1))
        wpool = ctx.enter_context(tc.tile_pool(name="wp", bufs=5))
        opool = ctx.enter_context(tc.tile_pool(name="outs", bufs=12))
        cpool = ctx.enter_context(tc.tile_pool(name="consts", bufs=1))
        psum = ctx.enter_context(tc.tile_pool(name="ps", bufs=5, space="PSUM"))
        tpsum = ctx.enter_context(tc.tile_pool(name="tp", bufs=3, space="PSUM"))

        ident = cpool.tile([P, P], FP8)
        make_identity(nc, ident[:])

        # warm the PE pstate ramp: ~4us of dummy transposes while the first
        # input loads are still in flight (full clock needs 3us busy)
        for _wu in range(6):
            wtp = tpsum.tile([P, KBT, 2 * P], FP8, tag="tp", name=f"wu{_wu}")
            for j in range(KBT):
                tpv = wtp[:, j, :]
                tp2 = bass.AP(tensor=tpv.tensor, offset=tpv.offset,
                              ap=[tpv.ap[0], [2, P]])
                nc.tensor.transpose(tp2, ident[:], ident[:])

        # bias staged [P, NE]: col e, part p = bias[e*128 + p]
        bias_sb = cpool.tile([P, NE], F32)
        nc.sync.dma_start(
            out=bias_sb[:],
            in_=bass.AP(tensor=bias_s.tensor, offset=bias_s.offset,
                        ap=[[1, P], [P, NE]]),
        )

        # x plane caches, fully resident: [128 k, plane, KB, M_C] fp8
        x_c = xcache.tile([P, 2, KB, M_C], FP8, name="x_c")
        xh_c, xl_c = x_c[:, 0], x_c[:, 1]
        w_t = {}                # e -> slot tile [P, KB, 2*P]

        def quant_x(src_rows, scale, hi_pool=False):
            """Load one [128, XT] f32 x strip-chunk; hi (ACT or Pool) /
            lo (DVE) into separate contiguous fp8 planes."""
            xt = stage.tile([P, XT], F32, tag="xt")
            nc.sync.dma_start(out=xt[:], in_=src_rows)
            t8 = q8.tile([P, 2, XT], FP8, tag="t8")
            th, tl = t8[:, 0, :], t8[:, 1, :]
            if hi_pool:
                nc.gpsimd.tensor_scalar(out=th, in0=xt[:],
                                        scalar1=float(scale), scalar2=None,
                                        op0=mybir.AluOpType.mult)
            else:
                nc.scalar.activation(out=th, in_=xt[:],
                                     func=mybir.ActivationFunctionType.Copy,
                                     scale=float(scale))
            nc.vector._custom_dve(LO_OP, out=tl, in0=xt[:], in1=th,
                                  s0=float(scale), s1=0.0)
            return th, tl

        def quant_w_packed(src_rows, scale, hi_eng="pool"):
            """Load one [128, XT] f32 w strip-chunk; hi / lo byte-interleaved
            into one packed (wh,wl) u16-grid tile. hi rides the Pool in the
            fill (ACT/DVE are saturated there) and the DVE in steady state
            (shorter chain; Pool keeps only the output stores)."""
            xt = stage.tile([P, XT], F32, tag="xt")
            nc.sync.dma_start(out=xt[:], in_=src_rows)
            t8 = q8.tile([P, 2, XT], FP8, tag="t8")
            b = t8[:].rearrange("p a x -> p (a x)")
            th = bass.AP(tensor=b.tensor, offset=b.offset,
                         ap=[b.ap[0], [2, XT]])
            tl = bass.AP(tensor=b.tensor, offset=b.offset + 1,
                         ap=[b.ap[0], [2, XT]])
            if hi_eng == "pool":
                nc.gpsimd.tensor_scalar(out=th, in0=xt[:],
                                        scalar1=float(scale), scalar2=None,
                                        op0=mybir.AluOpType.mult)
            elif hi_eng == "act":
                nc.scalar.activation(out=th, in_=xt[:],
                                     func=mybir.ActivationFunctionType.Copy,
                                     scale=float(scale))
            else:
                nc.vector._custom_dve(HI_OP, out=th, in0=xt[:],
                                      s0=float(scale), s1=0.0)
            nc.vector._custom_dve(LO_OP, out=tl, in0=xt[:], in1=th,
                                  s0=float(scale), s1=0.0)
            return t8

        def prod_x(mbp):
            """Quantize + fp8-PE-transpose one m-block pair (4 strips x
            full K) into the x plane caches."""
            for s in range(4 * mbp, 4 * mbp + 4):
                for kt in range(NKT):
                    th, tl = quant_x(
                        xs[s * P:(s + 1) * P, kt * XT:(kt + 1) * XT], SX,
                        hi_pool=False)
                    for t, dst, ce in ((th, xh_c, "act"), (tl, xl_c, "dve")):
                        tp = tpsum.tile([P, KBT, 2 * P], FP8, tag="tp")
                        for j in range(KBT):
                            tpv = tp[:, j, :]
                            tp2 = bass.AP(tensor=tpv.tensor,
                                          offset=tpv.offset,
                                          ap=[tpv.ap[0], [2, P]])
                            nc.tensor.transpose(
                                tp2, t[:, j * P:(j + 1) * P], ident[:])
                        f = tp[:]
                        ina = bass.AP(tensor=f.tensor, offset=f.offset,
                                      ap=[f.ap[0], [2 * P, KBT], [2, P]])
                        outa = dst[:, kt * KBT:(kt + 1) * KBT,
                                   s * P:(s + 1) * P]
                        if ce == "dve":
                            nc.vector.tensor_copy(outa, ina)
                        else:
                            nc.scalar.activation(
                                out=outa, in_=ina,
                                func=mybir.ActivationFunctionType.Copy)

        def prod_w(e, hi_eng="pool"):
            """Quantize one w sixteenth packed and xbar it straight into a
            [128 k, KB, 2*128] slot (no copies)."""
            wt = wpool.tile([P, KB, 2 * P], FP8, tag="wt", name=f"wt{e}")
            w_t[e] = wt
            prev = None

            def emit_tp(t8, kt):
                d16 = wt[:].bitcast(U16)[:, kt * KBT:(kt + 1) * KBT, :]
                nc.sync.dma_start_transpose(d16, t8[:].bitcast(U16))

            for kt in range(NKT):
                t8 = quant_w_packed(
                    ws[e * P:(e + 1) * P, kt * XT:(kt + 1) * XT], SW,
                    hi_eng=hi_eng)
                if prev is not None:
                    emit_tp(*prev)
                prev = (t8, kt)
            emit_tp(*prev)

        def mov_ap(cache, ks, mb):
            d = cache
            return bass.AP(tensor=d.tensor,
                           offset=d.offset + 2 * ks * M_C + mb * MBW,
                           ap=[d.ap[0], [M_C, 2], [1, MBW]])

        def stat_ap(e, ks, plane):
            d = w_t[e][:]
            return bass.AP(tensor=d.tensor,
                           offset=d.offset + 2 * ks * (2 * P) + plane,
                           ap=[d.ap[0], [2 * P, 2], [2, P]])

        def wave2(e, mbp):
            """One full-bank psum [128 n, 2x256 m] (m-blocks 2*mbp, 2*mbp+1)
            over full K: 2 x 48 DoubleRow matmuls, single fused evict."""
            ps = psum.tile([P, 2, MBW], F32, tag="ps", name=f"ps_{e}_{mbp}")
            nmm = 3 * KS
            for h in range(2):
                mb = 2 * mbp + h
                i = 0
                for ks in range(KS):
                    for wp_, mc in ((0, xh_c), (0, xl_c), (1, xh_c)):
                        nc.tensor.matmul(
                            ps[:, h, :], stat_ap(e, ks, wp_),
                            mov_ap(mc, ks, mb),
                            start=(i == 0), stop=(i == nmm - 1), perf_mode=DR)
                        i += 1
            ev = opool.tile([P, 2 * MBW], F16, tag="ev")
            nc.scalar.activation(
                out=ev[:], in_=ps[:].rearrange("p a b -> p (a b)"),
                func=mybir.ActivationFunctionType.Identity,
                bias=bias_sb[:, e:e + 1], scale=float(OSCALE),
            )
            pending_stores.append((ev, e, mbp))

        pending_stores = []

        def flush_stores(eng=None):
            for ev, e, mbp in pending_stores:
                (eng or nc.gpsimd).dma_start(
                    out=outT[e * P:(e + 1) * P,
                             mbp * 2 * MBW:(mbp + 1) * 2 * MBW],
                    in_=ev[:])
            pending_stores.clear()

        # ---------------- emission ----------------
        # Fill: staircase over (e < FILL_E) x mbp while x streams in (the
        # fp8 PE transposes keep the otherwise-idle PE busy). Steady: the
        # remaining sixteenths stream through the slot pool, quant chains
        # prefetched a few slots ahead of their waves.
        NMBP = NMB // 2
        emitted = set()
        ready_e = 0
        ready_p = 0

        def emit_ready():
            for e in range(ready_e):
                for p in range(ready_p):
                    if (e, p) not in emitted:
                        emitted.add((e, p))
                        wave2(e, p)

        steps = []
        for i in range(max(FILL_E, NMBP)):
            if i < NMBP:
                steps.append(("x", i))
            if i < FILL_E:
                steps.append(("w", i))
        for kind, idx in steps:
            if kind == "x":
                prod_x(idx)
                ready_p = idx + 1
            else:
                prod_w(idx)
                ready_e = idx + 1
            flush_stores()
            emit_ready()
        nexte = FILL_E
        for e in range(FILL_E, NE):
            while nexte < min(e + 5, NE):
                prod_w(nexte)
                nexte += 1
            flush_stores()
            for p in range(NMBP):
                wave2(e, p)
        flush_stores()

    nc.compile()
    return nc


_PROGRAM_CACHE = {}


def _get_program(M_C, K, N_C):
    key = (M_C, K, N_C)
    if key not in _PROGRAM_CACHE:
        _PROGRAM_CACHE[key] = build_program(M_C, K, N_C)
    return _PROGRAM_CACHE[key]


LAST_RESULTS = None


def kernel(x, weight, bias):
    global LAST_RESULTS
    M_FULL, K = x.shape
    N_FULL = weight.shape[0]
    RB, CB = 4, 2
    M_C, N_C = M_FULL // RB, N_FULL // CB

    nc = _get_program(M_C, K, N_C)

    x = np.asarray(x, np.float32)
    weight = np.asarray(weight, np.float32)
    bias = np.asarray(bias, np.float32)

    in_maps = []
    blocks = []
    for r in range(RB):
        for c in range(CB):
            in_maps.append({
                "xs": np.ascontiguousarray(x[r * M_C:(r + 1) * M_C]),
                "ws": np.ascontiguousarray(weight[c * N_C:(c + 1) * N_C]),
                "bias_s": np.ascontiguousarray(bias[c * N_C:(c + 1) * N_C]),
            })
            blocks.append((r, c))

    import os
    trace = bool(int(os.environ.get("KERNEL_TRACE", "0")))
    res = run_bass_kernel_spmd(nc, in_maps, core_ids=list(range(len(in_maps))),
                               trace=trace)
    LAST_RESULTS = res

    out = np.empty((M_FULL, N_FULL), np.float32)
    for i, (r, c) in enumerate(blocks):
        out[r * M_C:(r + 1) * M_C, c * N_C:(c + 1) * N_C] = \
            res.results[i]["outT"].astype(np.float32).T
    return out


# revision 94
# speedup vs baseline: 1.0223x; 1.0092x over previous
"""BFLinear (block-floating-point linear) Trainium2 kernel — fp8 DoubleRow.

Full problem: x[8192,4096] f32, weight[4096,4096] f32, bias[4096] f32.
  out = set_mantissa(bfp8_32(x) @ bfp8_32(weight).T + bias, 16 bits)

Sharding across 8 NeuronCores: 4 row-blocks of x  x  2 col-blocks of weight.
Each core computes outT_shard [N_C, M_C] fp16, host reassembles/transposes.

Numerical scheme: instead of replicating the reference's 8-bit-mantissa BFP
exactly (which forces bf16 matmuls), each operand is split into a hi/lo pair
of fp8 e4m3 planes at a fixed power-of-2 scale:
    xh = fp8(x*SX),  xl = fp8(x*SX - xh)      (and same for w at SW)
    out ~= (xh@wh + xl@wh + xh@wl) / (SX*SW) + bias
The dropped xl@wl term and the second-level quantization add ~0.7% rel l2 on
top of the reference's own BFP noise (~0.9%), landing at 1.17e-2 — inside
the 2e-2 gate with ~1.7x margin.  The payoff: all three matmul terms run as
fp8 MatmulPerfMode.DoubleRow (256-deep contraction, 0.5 cycles per moving
row), so per-core PE matmul time is 328us where the bit-exact bf16 path
needs 437us.

Per-core pipeline (single Tile program, SPMD, measured 530941 ns):
  1. load: f32 [128,1024] strip-chunks on the SP HWDGE queue.
  2. quantize x: ACT casts the hi plane (Copy with scale), one custom DVE op
     computes the lo residual (Src0*SX - Src1) straight to fp8.
     quantize w: same but hi on gpsimd and both planes written
     byte-interleaved (wh,wl) into one packed u16-grid tile.
  3. transpose to k-major [k = 128*kb + partition] operand layouts:
       - x planes: fp8 PE identity transposes (the PE is otherwise idle
         during the x fill; PSUM holds fp8 at u16 spacing) + strided ACT/DVE
         copies compacting into the resident caches [128, 2, KB, M_C].
       - w: one DMA-xbar transpose per packed chunk straight into streamed
         slots [128, KB, 2*128] — no copies; the DoubleRow stationary reads
         wh/wl via inner stride-2 APs (wl at the odd byte offset), which the
         LdWeights ISA permits (the 16B slot-stride rule only binds the
         outer/slot dim, satisfied by the kb-pair stride).
  4. matmul: per (w-sixteenth e, m-block-pair) one full-bank PSUM
     [128, 2, 256] f32 accumulates 2 x 48 DoubleRow matmuls
     (16 k-supertiles x 3 terms), moving free = (2, 256).
  5. evict (ACT): fp16 with fused bias + 2^-14 descale, stores on the
     gpsimd SWDGE queue (emitted one production step behind their evicts).

Schedule: a short burst of dummy PE transposes warms the pstate ramp, then
a staircase over (e < 4) x m-block-pairs keeps the PE on transposes+waves
while x streams in; the remaining 12 sixteenths stream through a 5-slot
pool, quant chains prefetched ~4 slots ahead of their waves.  Deep evict
(12) and psum (5+3) pools keep the tile framework's buffer-recycle barriers
(EventSemaphores, which serialize whole engine queues) off the critical
path — shallow pools there previously cost >70us in cascaded stalls and
pstate resets.
"""

import re
from contextlib import ExitStack

import numpy as np

import concourse.bass as bass
import concourse.dve_ops as dve_ops
from concourse import bacc
import concourse.tile as tile
from concourse import mybir
from concourse.bass_utils import run_bass_kernel_spmd
from concourse.dve_spec import C0, Spec, Src0, Src1
from concourse.masks import make_identity

P = 128
F32 = mybir.dt.float32
F16 = mybir.dt.float16
FP8 = mybir.dt.float8e4
U16 = mybir.dt.uint16
U32 = mybir.dt.uint32

SX = 32.0      # |x| < 7.5 -> |xh| < 240 (e4m3 max)
SW = 512.0     # |w| < 0.11 -> |wh| < 56.3
OSCALE = 1.0 / (SX * SW)


# --------------------------------------------------------------------------
# custom DVE op: out_fp8 = Src0 * C0 - Src1   (lo-plane residual)
# --------------------------------------------------------------------------
def _lo_ref(in0, in1, s0, s1, imm2):
    g = np.asarray(in0, np.float32)
    h = np.asarray(in1, np.float32)
    return (g * np.float32(s0) - h).astype(np.float32)


def _hi_ref(in0, in1, s0, s1, imm2):
    g = np.asarray(in0, np.float32)
    return (g * np.float32(s0)).astype(np.float32)


def _register_op(name, spec):
    for existing in dve_ops.OPS:
        if existing.name == name:
            return existing
    ver = "v3"
    op = dve_ops.DveOp(name, spec, subdim=False, uops_sha={})
    dve_ops.OPS.append(op)
    dve_ops._SUB_OPCODE_FOR_NAME[name] = (
        dve_ops._CUSTOM_DVE_ROW_BASE + len(dve_ops.OPS) - 1
    )
    dve_ops.CUSTOM_DVE_SPECS[name] = spec
    try:
        op.compile(ver)
    except ValueError as e:
        m = re.search(r'uops_sha\["v3"\]="([0-9a-f]+)"', str(e))
        if not m:
            raise
        op = dve_ops.DveOp(name, spec, subdim=False, uops_sha={ver: m.group(1)})
        dve_ops.OPS[-1] = op
    op.compile(ver)
    return op


LO_OP = _register_op("FP8_LO_RESID_ANT", Spec(body=Src0 * C0 - Src1,
                                              reference=_lo_ref))
HI_OP = _register_op("FP8_HI_SCALE_ANT", Spec(body=Src0 * C0,
                                              reference=_hi_ref))


# --------------------------------------------------------------------------
# program builder
# --------------------------------------------------------------------------
def build_program(M_C, K, N_C, num_devices=1):
    """One SPMD core program: xs [M_C,K], ws [N_C,K], bias_s [N_C]
    -> outT [N_C, M_C] fp16.

    Operand layout (both sides k-major, one k per partition,
    k = 128*kb + partition):
      x planes: [128, 2(plane), KB, M_C] fp8, filled by fp8 PE identity
        transposes (PSUM holds fp8 at u16 spacing; a strided ACT/DVE copy
        compacts into the cache).
      w slots:  [128, KB, 2*128] fp8 with wh/wl byte-interleaved along n,
        filled by the DMA xbar from a (wh,wl)-packed u16 quant tile; the
        DoubleRow stationary reads it with an inner stride-2 AP (wl at the
        odd byte offset), which the ISA allows for LdWeights.
    DoubleRow slot pairs are adjacent kb blocks (stride 16B-aligned as the
    ISA requires); each matmul contracts 256 k at 0.5 cycles/row.
    """
    XT = 1024                   # staged strip k-width
    NKT = K // XT               # k-chunks per strip (4)
    KB = K // P                 # k blocks (32)
    KBT = XT // P               # k blocks per staged chunk (8)
    KS = K // 256               # DoubleRow k-supertiles (16)
    NE = N_C // P               # w sixteenths (16)
    MBW = 256                   # m-block width
    NMB = M_C // MBW            # m-blocks (8)
    FILL_E = min(1, N_C // P)   # w slots resident through x fill
    DR = mybir.MatmulPerfMode.DoubleRow

    nc = bacc.Bacc("TRN2", target_bir_lowering=False, debug=False,
                   enable_asserts=True, num_devices=num_devices)
    xs = nc.dram_tensor("xs", [M_C, K], F32, kind="ExternalInput").ap()
    ws = nc.dram_tensor("ws", [N_C, K], F32, kind="ExternalInput").ap()
    bias_s = nc.dram_tensor("bias_s", [N_C], F32, kind="ExternalInput").ap()
    outT = nc.dram_tensor("outT", [N_C, M_C], F16, kind="ExternalOutput").ap()

    with tile.TileContext(nc) as tc, ExitStack() as ctx:
        stage = ctx.enter_context(tc.tile_pool(name="stage", bufs=5))
        q8 = ctx.enter_context(tc.tile_pool(name="q8", bufs=3))
        xcache = ctx.enter_context(tc.tile_pool(name="xc", bufs=1))
        wpool = ctx.enter_context(tc.tile_pool(name="wp", bufs=5))
        opool = ctx.enter_context(tc.tile_pool(name="outs", bufs=12))
        cpool = ctx.enter_context(tc.tile_pool(name="consts", bufs=1))
        psum = ctx.enter_context(tc.tile_pool(name="ps", bufs=5, space="PSUM"))
        tpsum = ctx.enter_context(tc.tile_pool(name="tp", bufs=3, space="PSUM"))

        ident = cpool.tile([P, P], FP8)
        make_identity(nc, ident[:])

        # warm the PE pstate ramp: ~4us of dummy transposes while the first
        # input loads are still in flight (full clock needs 3us busy)
        for _wu in range(6):
            wtp = tpsum.tile([P, KBT, 2 * P], FP8, tag="tp", name=f"wu{_wu}")
            for j in range(KBT):
                tpv = wtp[:, j, :]
                tp2 = bass.AP(tensor=tpv.tensor, offset=tpv.offset,
                              ap=[tpv.ap[0], [2, P]])
                nc.tensor.transpose(tp2, ident[:], ident[:])

        # bias staged [P, NE]: col e, part p = bias[e*128 + p]
        bias_sb = cpool.tile([P, NE], F32)
        nc.sync.dma_start(
            out=bias_sb[:],
            in_=bass.AP(tensor=bias_s.tensor, offset=bias_s.offset,
                        ap=[[1, P], [P, NE]]),
        )

        # x plane caches, fully resident: [128 k, plane, KB, M_C] fp8
        x_c = xcache.tile([P, 2, KB, M_C], FP8, name="x_c")
        xh_c, xl_c = x_c[:, 0], x_c[:, 1]
        w_t = {}                # e -> slot tile [P, KB, 2*P]

        def quant_x(src_rows, scale, hi_pool=False):
            """Load one [128, XT] f32 x strip-chunk; hi (ACT or Pool) /
            lo (DVE) into separate contiguous fp8 planes."""
            xt = stage.tile([P, XT], F32, tag="xt")
            nc.sync.dma_start(out=xt[:], in_=src_rows)
            t8 = q8.tile([P, 2, XT], FP8, tag="t8")
            th, tl = t8[:, 0, :], t8[:, 1, :]
            if hi_pool:
                nc.gpsimd.tensor_scalar(out=th, in0=xt[:],
                                        scalar1=float(scale), scalar2=None,
                                        op0=mybir.AluOpType.mult)
            else:
                nc.scalar.activation(out=th, in_=xt[:],
                                     func=mybir.ActivationFunctionType.Copy,
                                     scale=float(scale))
            nc.vector._custom_dve(LO_OP, out=tl, in0=xt[:], in1=th,
                                  s0=float(scale), s1=0.0)
            return th, tl

        def quant_w_packed(src_rows, scale, hi_eng="pool"):
            """Load one [128, XT] f32 w strip-chunk; hi / lo byte-interleaved
            into one packed (wh,wl) u16-grid tile. hi rides the Pool in the
            fill (ACT/DVE are saturated there) and the DVE in steady state
            (shorter chain; Pool keeps only the output stores)."""
            xt = stage.tile([P, XT], F32, tag="xt")
            nc.sync.dma_start(out=xt[:], in_=src_rows)
            t8 = q8.tile([P, 2, XT], FP8, tag="t8")
            b = t8[:].rearrange("p a x -> p (a x)")
            th = bass.AP(tensor=b.tensor, offset=b.offset,
                         ap=[b.ap[0], [2, XT]])
            tl = bass.AP(tensor=b.tensor, offset=b.offset + 1,
                         ap=[b.ap[0], [2, XT]])
            if hi_eng == "pool":
                nc.gpsimd.tensor_scalar(out=th, in0=xt[:],
                                        scalar1=float(scale), scalar2=None,
                                        op0=mybir.AluOpType.mult)
            elif hi_eng == "act":
                nc.scalar.activation(out=th, in_=xt[:],
                                     func=mybir.ActivationFunctionType.Copy,
                                     scale=float(scale))
            else:
                nc.vector._custom_dve(HI_OP, out=th, in0=xt[:],
                                      s0=float(scale), s1=0.0)
            nc.vector._custom_dve(LO_OP, out=tl, in0=xt[:], in1=th,
                                  s0=float(scale), s1=0.0)
            return t8

        def prod_x(mbp):
            """Quantize + fp8-PE-transpose one m-block pair (4 strips x
            full K) into the x plane caches."""
            for s in range(4 * mbp, 4 * mbp + 4):
                for kt in range(NKT):
                    th, tl = quant_x(
                        xs[s * P:(s + 1) * P, kt * XT:(kt + 1) * XT], SX,
                        hi_pool=False)
                    for t, dst, ce in ((th, xh_c, "act"), (tl, xl_c, "dve")):
                        tp = tpsum.tile([P, KBT, 2 * P], FP8, tag="tp")
                        for j in range(KBT):
                            tpv = tp[:, j, :]
                            tp2 = bass.AP(tensor=tpv.tensor,
                                          offset=tpv.offset,
                                          ap=[tpv.ap[0], [2, P]])
                            nc.tensor.transpose(
                                tp2, t[:, j * P:(j + 1) * P], ident[:])
                        f = tp[:]
                        ina = bass.AP(tensor=f.tensor, offset=f.offset,
                                      ap=[f.ap[0], [2 * P, KBT], [2, P]])
                        outa = dst[:, kt * KBT:(kt + 1) * KBT,
                                   s * P:(s + 1) * P]
                        if ce == "dve":
                            nc.vector.tensor_copy(outa, ina)
                        else:
                            nc.scalar.activation(
                                out=outa, in_=ina,
                                func=mybir.ActivationFunctionType.Copy)

        def prod_w(e, hi_eng="pool"):
            """Quantize one w sixteenth packed and xbar it straight into a
            [128 k, KB, 2*128] slot (no copies)."""
            wt = wpool.tile([P, KB, 2 * P], FP8, tag="wt", name=f"wt{e}")
            w_t[e] = wt
            prev = None

            def emit_tp(t8, kt):
                d16 = wt[:].bitcast(U16)[:, kt * KBT:(kt + 1) * KBT, :]
                nc.sync.dma_start_transpose(d16, t8[:].bitcast(U16))

            for kt in range(NKT):
                t8 = quant_w_packed(
                    ws[e * P:(e + 1) * P, kt * XT:(kt + 1) * XT], SW,
                    hi_eng=hi_eng)
                if prev is not None:
                    emit_tp(*prev)
                prev = (t8, kt)
            emit_tp(*prev)

        def mov_ap(cache, ks, mb):
            d = cache
            return bass.AP(tensor=d.tensor,
                           offset=d.offset + 2 * ks * M_C + mb * MBW,
                           ap=[d.ap[0], [M_C, 2], [1, MBW]])

        def stat_ap(e, ks, plane):
            d = w_t[e][:]
            return bass.AP(tensor=d.tensor,
                           offset=d.offset + 2 * ks * (2 * P) + plane,
                           ap=[d.ap[0], [2 * P, 2], [2, P]])

        def wave2(e, mbp):
            """One full-bank psum [128 n, 2x256 m] (m-blocks 2*mbp, 2*mbp+1)
            over full K: 2 x 48 DoubleRow matmuls, single fused evict."""
            ps = psum.tile([P, 2, MBW], F32, tag="ps", name=f"ps_{e}_{mbp}")
            nmm = 3 * KS
            for h in range(2):
                mb = 2 * mbp + h
                i = 0
                for ks in range(KS):
                    for wp_, mc in ((0, xh_c), (0, xl_c), (1, xh_c)):
                        nc.tensor.matmul(
                            ps[:, h, :], stat_ap(e, ks, wp_),
                            mov_ap(mc, ks, mb),
                            start=(i == 0), stop=(i == nmm - 1), perf_mode=DR)
                        i += 1
            ev = opool.tile([P, 2 * MBW], F16, tag="ev")
            nc.scalar.activation(
                out=ev[:], in_=ps[:].rearrange("p a b -> p (a b)"),
                func=mybir.ActivationFunctionType.Identity,
                bias=bias_sb[:, e:e + 1], scale=float(OSCALE),
            )
            pending_stores.append((ev, e, mbp))

        pending_stores = []

        def flush_stores(eng=None):
            for ev, e, mbp in pending_stores:
                (eng or nc.gpsimd).dma_start(
                    out=outT[e * P:(e + 1) * P,
                             mbp * 2 * MBW:(mbp + 1) * 2 * MBW],
                    in_=ev[:])
            pending_stores.clear()

        # ---------------- emission ----------------
        # Fill: staircase over (e < FILL_E) x mbp while x streams in (the
        # fp8 PE transposes keep the otherwise-idle PE busy). Steady: the
        # remaining sixteenths stream through the slot pool, quant chains
        # prefetched a few slots ahead of their waves.
        NMBP = NMB // 2
        emitted = set()
        ready_e = 0
        ready_p = 0

        def emit_ready():
            for e in range(ready_e):
                for p in range(ready_p):
                    if (e, p) not in emitted:
                        emitted.add((e, p))
                        wave2(e, p)

        steps = []
        for i in range(max(FILL_E, NMBP)):
            if i < NMBP:
                steps.append(("x", i))
            if i < FILL_E:
                steps.append(("w", i))
        for kind, idx in steps:
            if kind == "x":
                prod_x(idx)
                ready_p = idx + 1
            else:
                prod_w(idx)
                ready_e = idx + 1
            flush_stores()
            emit_ready()
        nexte = FILL_E
        for e in range(FILL_E, NE):
            while nexte < min(e + 5, NE):
                prod_w(nexte)
                nexte += 1
            flush_stores()
            for p in range(NMBP):
                wave2(e, p)
        flush_stores()

    nc.compile()
    return nc


_PROGRAM_CACHE = {}


def _get_program(M_C, K, N_C):
    key = (M_C, K, N_C)
    if key not in _PROGRAM_CACHE:
        _PROGRAM_CACHE[key] = build_program(M_C, K, N_C)
    return _PROGRAM_CACHE[key]


LAST_RESULTS = None


def kernel(x, weight, bias):
    global LAST_RESULTS
    M_FULL, K = x.shape
    N_FULL = weight.shape[0]
    RB, CB = 4, 2
    M_C, N_C = M_FULL // RB, N_FULL // CB

    nc = _get_program(M_C, K, N_C)

    x = np.asarray(x, np.float32)
    weight = np.asarray(weight, np.float32)
    bias = np.asarray(bias, np.float32)

    in_maps = []
    blocks = []
    for r in range(RB):
        for c in range(CB):
            in_maps.append({
                "xs": np.ascontiguousarray(x[r * M_C:(r + 1) * M_C]),
                "ws": np.ascontiguousarray(weight[c * N_C:(c + 1) * N_C]),
                "bias_s": np.ascontiguousarray(bias[c * N_C:(c + 1) * N_C]),
            })
            blocks.append((r, c))

    import os
    trace = bool(int(os.environ.get("KERNEL_TRACE", "0")))
    res = run_bass_kernel_spmd(nc, in_maps, core_ids=list(range(len(in_maps))),
                               trace=trace)
    LAST_RESULTS = res

    out = np.empty((M_FULL, N_FULL), np.float32)
    for i, (r, c) in enumerate(blocks):
        out[r * M_C:(r + 1) * M_C, c * N_C:(c + 1) * N_C] = \
            res.results[i]["outT"].astype(np.float32).T
    return out
